# revision 2
# baseline (speedup 1.0000x reference)
"""Trainium2 Bass kernel: fused MoE-routing block (nn_CLVBase_75282186764626).

Math (per row of B=262144):
    e      = relu(concat(s_e, p_e) @ W_fuse + b_fuse)          [256]
    g      = tanh(e @ W_a + b_a)                               [256]
    lg     = g @ W_smax          (router logits)               [8]
    smoid  = sigmoid(g @ W_smoid)                              [8]
    probs  = softmax(lg)                                       [8]
    out    = C[argmax(lg)] + b_out,  C_k = e[32k:32k+32] @ W_out[32k:32k+32]

Strategy: pure data parallel over 8 NeuronCores. On host, inputs are split
into fp16 hi/lo planes (x == xh + xl to ~21 bits). The bulk kernel runs
1-pass fp16 matmuls (fast path) and also emits each row's top-2 logit
values; rows whose top-2 gap is below a threshold (where the 1-pass argmax
could disagree with the f32 reference) are re-run through a small 3-pass
fp16 kernel (xh@Wh + xl@Wh + xh@Wl, ~1e-6 accurate) and patched in.
Activations stay feature-major on chip (PE transposes the fp16 inputs);
ACT does tanh/exp/sigmoid; DVE does relu/copies/softmax pieces.
"""

import os
import numpy as np

import jax

import concourse.bass as bass
import concourse.tile as tile
import concourse.mybir as mybir
from concourse.vector_clock import ScopedClock
from concourse.masks import make_identity
from concourse.bass2jax import (
    _bass_exec_p,
    install_neuronx_cc_hook,
    partition_id_tensor,
)
from jax.sharding import Mesh, PartitionSpec
from jax.experimental.shard_map import shard_map

F32 = mybir.dt.float32
FP16 = mybir.dt.float16
AF = mybir.ActivationFunctionType
OP = mybir.AluOpType

B_TOTAL = 262144
N_CORES = 8
R_BULK = B_TOTAL // N_CORES       # 32768 rows per core
THETA = 4e-3                      # top-2 gap threshold for the repair pass
REP_CAP = 2048                    # repair rows per core per launch (16k total)

# ----------------------------------------------------------------------------
# Workarounds for the public walrus in this container: it encodes at most ONE
# sync wait per instruction. Split extra waits onto NOPs / chained drains.
# ----------------------------------------------------------------------------


def _drain_and_barrier(self, tick_clock, wait_clock):
    nc = self.nc
    drain_inst = nc.sync.drain()
    wait_clock.add_sem_waits(
        drain_inst.ins, ScopedClock({None: tick_clock.global_clock})
    )
    si = drain_inst.ins.sync_info
    waits = list(si.on_wait) if si is not None else []
    if len(waits) > 1:
        drain_inst.ins.sync_info = mybir.SyncInfo(
            on_update=list(si.on_update), on_wait=[waits[0]]
        )
        for w in waits[1:]:
            extra = nc.sync.drain()
            extra.ins.sync_info = mybir.SyncInfo(on_update=[], on_wait=[w])
    nc.all_engine_barrier()
    assert self.sems is not None
    popped = nc._tile_sem_poison_stack.pop()
    assert popped is self._sem_poison
    nc.clear_and_free_semaphores(list(self.sems.allocated().values()))
    nc.all_engine_barrier()


tile.TileContext._drain_and_barrier = _drain_and_barrier


def _split_multi_waits(nc):
    n_split = 0
    for f in nc.m.functions:
        for bb in f.blocks:
            if not any(
                inst.sync_info is not None and len(inst.sync_info.on_wait) > 1
                for inst in bb.instructions
            ):
                continue
            new_list = []
            for inst in bb.instructions:
                si = inst.sync_info
                if si is not None and len(si.on_wait) > 1:
                    waits = list(si.on_wait)
                    for k, w in enumerate(waits[:-1]):
                        new_list.append(
                            mybir.InstNoOp(
                                name=f"{inst.name}-sw{k}",
                                engine=inst.engine,
                                bass_nofuse=True,
                                sync_info=mybir.SyncInfo(on_wait=[w], on_update=[]),
                            )
                        )
                    inst.sync_info = mybir.SyncInfo(
                        on_wait=[waits[-1]], on_update=list(si.on_update)
                    )
                    n_split += 1
                new_list.append(inst)
            bb.instructions = new_list
    return n_split


# ----------------------------------------------------------------------------
# PJRT runner: build the jitted callable once per program, reuse across calls.
# ----------------------------------------------------------------------------


def _make_runner(nc, n_cores):
    install_neuronx_cc_hook()
    partition_name = nc.partition_id_tensor.name if nc.partition_id_tensor else None

    in_names, out_names, out_avals, zero_shapes = [], [], [], []
    for alloc in nc.m.functions[0].allocations:
        if not isinstance(alloc, mybir.MemoryLocationSet):
            continue
        name = alloc.memorylocations[0].name
        if alloc.kind == "ExternalInput":
            if name != partition_name:
                in_names.append(name)
        elif alloc.kind == "ExternalOutput":
            out_names.append(name)
            shape = tuple(alloc.tensor_shape)
            dtype = mybir.dt.np(alloc.dtype)
            out_avals.append(jax.core.ShapedArray(shape, dtype))
            zero_shapes.append((shape, dtype))
    n_params = len(in_names)
    n_outs = len(out_avals)
    all_in_names = list(in_names) + list(out_names)
    if partition_name is not None:
        all_in_names.append(partition_name)
    donate = tuple(range(n_params, n_params + n_outs))

    def _body(*args):
        operands = list(args)
        if partition_name is not None:
            operands.append(partition_id_tensor())
        outs = _bass_exec_p.bind(
            *operands,
            out_avals=tuple(out_avals),
            in_names=tuple(all_in_names),
            out_names=tuple(out_names),
            lowering_input_output_aliases=(),
            sim_require_finite=True,
            sim_require_nnan=True,
            nc=nc,
        )
        return tuple(outs)

    devices = jax.devices()[:n_cores]
    mesh = Mesh(np.asarray(devices), ("core",))
    in_specs = (PartitionSpec("core"),) * (n_params + n_outs)
    out_specs = (PartitionSpec("core"),) * len(out_names)
    jit_fn = jax.jit(
        shard_map(
            _body, mesh=mesh, in_specs=in_specs, out_specs=out_specs, check_rep=False
        ),
        donate_argnums=donate,
        keep_unused=True,
    )

    def call(in_maps, block=True):
        assert len(in_maps) == n_cores
        args = [
            np.ascontiguousarray(
                np.concatenate([np.asarray(m[n]) for m in in_maps], axis=0)
            )
            for n in in_names
        ]
        args += [
            np.zeros((n_cores * s[0], *s[1:]), d) for (s, d) in zero_shapes
        ]
        outs = jit_fn(*args)
        if block:
            jax.block_until_ready(outs)
        results = []
        for c in range(n_cores):
            d = {}
            for i, nm in enumerate(out_names):
                arr = outs[i]
                per = arr.shape[0] // n_cores
                d[nm] = np.asarray(arr[c * per:(c + 1) * per])
            results.append(d)
        return results

    return call


# ----------------------------------------------------------------------------
# Kernel builder
# ----------------------------------------------------------------------------


def build_moe(nrows, n_pass, rep=1):
    """One-core program; run SPMD on 8 cores. nrows % 512 == 0.

    Row layout: partition p owns rows [p*rpp, (p+1)*rpp); iteration i
    handles within-partition rows [4i, 4i+4). Global local-row index of
    (p, i, q) is p*rpp + 4i + q. `rep` wraps the whole body in a hardware
    loop (timing only — recomputes identical outputs).
    """
    iters = nrows // 512
    rpp = nrows // 128
    nc = bass.Bass()

    sh_d = nc.dram_tensor("sh", [nrows, 256], FP16, kind="ExternalInput")
    ph_d = nc.dram_tensor("ph", [nrows, 256], FP16, kind="ExternalInput")
    if n_pass == 3:
        sl_d = nc.dram_tensor("sl", [nrows, 256], FP16, kind="ExternalInput")
        pl_d = nc.dram_tensor("pl", [nrows, 256], FP16, kind="ExternalInput")
    wfh_d = nc.dram_tensor("wfh", [512, 256], FP16, kind="ExternalInput")
    wah_d = nc.dram_tensor("wah", [256, 256], FP16, kind="ExternalInput")
    whd_d = nc.dram_tensor("whd", [256, 40], FP16, kind="ExternalInput")
    if n_pass == 3:
        wfl_d = nc.dram_tensor("wfl", [512, 256], FP16, kind="ExternalInput")
        wal_d = nc.dram_tensor("wal", [256, 256], FP16, kind="ExternalInput")
        whl_d = nc.dram_tensor("whl", [256, 40], FP16, kind="ExternalInput")
    bf_d = nc.dram_tensor("bf", [128, 2], F32, kind="ExternalInput")
    ba_d = nc.dram_tensor("ba", [128, 2], F32, kind="ExternalInput")
    bo_d = nc.dram_tensor("bo", [128, 1], F32, kind="ExternalInput")
    id16_d = nc.dram_tensor("id16", [128, 128], FP16, kind="ExternalInput")

    out_d = nc.dram_tensor("out", [nrows, 1], F32, kind="ExternalOutput")
    probs_d = nc.dram_tensor("probs", [nrows, 8], F32, kind="ExternalOutput")
    smoid_d = nc.dram_tensor("smoid", [nrows, 8], F32, kind="ExternalOutput")
    top2_d = nc.dram_tensor("top2", [nrows, 2], F32, kind="ExternalOutput")

    with tile.TileContext(nc) as tc:
        with (
            tc.tile_pool(name="wp", bufs=1) as wp,
            tc.tile_pool(name="lp", bufs=3) as lp,
            tc.tile_pool(name="xp", bufs=2) as xp,
            tc.tile_pool(name="ep", bufs=2) as ep,
            tc.tile_pool(name="sp", bufs=2) as sp,
            tc.tile_pool(name="op_", bufs=2) as op_,
            tc.tile_pool(name="pxt", bufs=1, space="PSUM") as pxt,
            tc.tile_pool(name="pe_", bufs=1, space="PSUM") as pe_,
            tc.tile_pool(name="pg_", bufs=1, space="PSUM") as pg_,
            tc.tile_pool(name="pch", bufs=1, space="PSUM") as pch,
            tc.tile_pool(name="pbt", bufs=1, space="PSUM") as pbt,
        ):
            # ---- constants / weights (loaded once) ----
            wfh_s = wp.tile([128, 4, 256], FP16, tag="wfh")
            nc.sync.dma_start(out=wfh_s, in_=wfh_d.rearrange("(k p) m -> p k m", p=128))
            wah_s = wp.tile([128, 2, 256], FP16, tag="wah")
            nc.sync.dma_start(out=wah_s, in_=wah_d.rearrange("(k p) m -> p k m", p=128))
            whd_s = wp.tile([128, 2, 40], FP16, tag="whd")
            nc.sync.dma_start(out=whd_s, in_=whd_d.rearrange("(k p) m -> p k m", p=128))
            if n_pass == 3:
                wfl_s = wp.tile([128, 4, 256], FP16, tag="wfl")
                nc.sync.dma_start(out=wfl_s, in_=wfl_d.rearrange("(k p) m -> p k m", p=128))
                wal_s = wp.tile([128, 2, 256], FP16, tag="wal")
                nc.sync.dma_start(out=wal_s, in_=wal_d.rearrange("(k p) m -> p k m", p=128))
                whl_s = wp.tile([128, 2, 40], FP16, tag="whl")
                nc.sync.dma_start(out=whl_s, in_=whl_d.rearrange("(k p) m -> p k m", p=128))
            bf_s = wp.tile([128, 2], F32, tag="bf")
            nc.sync.dma_start(out=bf_s, in_=bf_d[:, :])
            ba_s = wp.tile([128, 2], F32, tag="ba")
            nc.sync.dma_start(out=ba_s, in_=ba_d[:, :])
            bo_s = wp.tile([128, 1], F32, tag="bo")
            nc.sync.dma_start(out=bo_s, in_=bo_d[:, :])
            id16 = wp.tile([128, 128], FP16, tag="id16")
            nc.sync.dma_start(out=id16, in_=id16_d[:, :])
            id32 = wp.tile([128, 128], F32, tag="id32")
            make_identity(nc, id32)

            def body():
                for i in range(iters):
                    # ---- input loads (2KB contiguous per partition) ----
                    sh_t = lp.tile([128, 4, 256], FP16, tag="sh")
                    nc.sync.dma_start(
                        out=sh_t,
                        in_=bass.AP(
                            tensor=sh_d, offset=i * 4 * 256,
                            ap=[[rpp * 256, 128], [256, 4], [1, 256]],
                        ),
                    )
                    ph_t = lp.tile([128, 4, 256], FP16, tag="ph")
                    nc.sync.dma_start(
                        out=ph_t,
                        in_=bass.AP(
                            tensor=ph_d, offset=i * 4 * 256,
                            ap=[[rpp * 256, 128], [256, 4], [1, 256]],
                        ),
                    )
                    if n_pass == 3:
                        sl_t = lp.tile([128, 4, 256], FP16, tag="sl")
                        nc.sync.dma_start(
                            out=sl_t,
                            in_=bass.AP(
                                tensor=sl_d, offset=i * 4 * 256,
                                ap=[[rpp * 256, 128], [256, 4], [1, 256]],
                            ),
                        )
                        pl_t = lp.tile([128, 4, 256], FP16, tag="pl")
                        nc.sync.dma_start(
                            out=pl_t,
                            in_=bass.AP(
                                tensor=pl_d, offset=i * 4 * 256,
                                ap=[[rpp * 256, 128], [256, 4], [1, 256]],
                            ),
                        )

                    # ---- transpose x to feature-major: xts[f, col] ----
                    def transpose_plane(s_src, p_src, tag):
                        ps = pxt.tile([128, 4, 512], FP16, tag="pxt")
                        for f in range(4):
                            src = s_src if f < 2 else p_src
                            fs = (f % 2) * 128
                            for q in range(4):
                                first = (f % 2 == 0) and (q == 0)
                                last = (f % 2 == 1) and (q == 3)
                                nc.tensor.matmul(
                                    ps[:, f, q * 128:(q + 1) * 128],
                                    lhsT=src[:, q, fs:fs + 128],
                                    rhs=id16,
                                    is_transpose=True,
                                    start=first, stop=last,
                                    skip_group_check=True,
                                )
                        xts = xp.tile([128, 4, 512], FP16, tag=tag)
                        nc.scalar.copy(xts[:, 0:2, :], ps[:, 0:2, :])
                        nc.vector.tensor_copy(xts[:, 2:4, :], ps[:, 2:4, :])
                        return xts

                    xts = transpose_plane(sh_t, ph_t, "xts")
                    if n_pass == 3:
                        xls = transpose_plane(sl_t, pl_t, "xls")

                    # ---- fuse matmul -> e (feature-major [256, 512]) ----
                    pe_t = pe_.tile([128, 2, 512], F32, tag="pe")
                    for m in range(2):
                        terms = [(wfh_s, xts)]
                        if n_pass == 3:
                            terms += [(wfh_s, xls), (wfl_s, xts)]
                        n_mm = 4 * len(terms)
                        j = 0
                        for (wt, xt) in terms:
                            for k in range(4):
                                nc.tensor.matmul(
                                    pe_t[:, m, :],
                                    lhsT=wt[:, k, m * 128:(m + 1) * 128],
                                    rhs=xt[:, k, :],
                                    start=(j == 0), stop=(j == n_mm - 1),
                                )
                                j += 1

                    # ---- relu(+bias) -> eh (fp16), el (3-pass) ----
                    eh = ep.tile([128, 2, 512], FP16, tag="eh")
                    if n_pass == 1:
                        for m in range(2):
                            nc.vector.tensor_scalar(
                                eh[:, m, :], pe_t[:, m, :],
                                scalar1=bf_s[:, m:m + 1], scalar2=0.0,
                                op0=OP.add, op1=OP.max,
                            )
                    else:
                        e32 = ep.tile([128, 2, 512], F32, tag="e32")
                        el = ep.tile([128, 2, 512], FP16, tag="el")
                        for m in range(2):
                            nc.vector.tensor_scalar(
                                e32[:, m, :], pe_t[:, m, :],
                                scalar1=bf_s[:, m:m + 1], scalar2=0.0,
                                op0=OP.add, op1=OP.max,
                            )
                            nc.scalar.copy(eh[:, m, :], e32[:, m, :])
                            nc.vector.scalar_tensor_tensor(
                                el[:, m, :], in0=eh[:, m, :], scalar=-1.0,
                                in1=e32[:, m, :],
                                op0=OP.mult, op1=OP.add,
                            )

                    # ---- C = e @ W_blk (1-pass) into pch[32:40] ----
                    pch_t = pch.tile([40, 512], F32, tag="pch")
                    for k in range(2):
                        nc.tensor.matmul(
                            pch_t[32:40, :],
                            lhsT=whd_s[:, k, 32:40],
                            rhs=eh[:, k, :],
                            start=(k == 0), stop=(k == 1),
                            skip_group_check=True,
                        )

                    # ---- W_a matmul -> g ----
                    pg_t = pg_.tile([128, 2, 512], F32, tag="pg")
                    for m in range(2):
                        terms = [(wah_s, eh)]
                        if n_pass == 3:
                            terms += [(wah_s, el), (wal_s, eh)]
                        n_mm = 2 * len(terms)
                        j = 0
                        for (wt, xt) in terms:
                            for k in range(2):
                                nc.tensor.matmul(
                                    pg_t[:, m, :],
                                    lhsT=wt[:, k, m * 128:(m + 1) * 128],
                                    rhs=xt[:, k, :],
                                    start=(j == 0), stop=(j == n_mm - 1),
                                )
                                j += 1

                    # ---- tanh(+bias) -> gh (fp16), gl (3-pass) ----
                    gh = ep.tile([128, 2, 512], FP16, tag="gh")
                    if n_pass == 1:
                        for m in range(2):
                            nc.scalar.activation(
                                gh[:, m, :], pg_t[:, m, :], AF.Tanh,
                                bias=ba_s[:, m:m + 1],
                            )
                    else:
                        g32 = ep.tile([128, 2, 512], F32, tag="g32")
                        gl = ep.tile([128, 2, 512], FP16, tag="gl")
                        for m in range(2):
                            nc.scalar.activation(
                                g32[:, m, :], pg_t[:, m, :], AF.Tanh,
                                bias=ba_s[:, m:m + 1],
                            )
                            nc.vector.tensor_copy(gh[:, m, :], g32[:, m, :])
                            nc.vector.scalar_tensor_tensor(
                                gl[:, m, :], in0=gh[:, m, :], scalar=-1.0,
                                in1=g32[:, m, :],
                                op0=OP.mult, op1=OP.add,
                            )

                    # ---- heads matmul -> pch[0:16] ----
                    terms = [(whd_s, gh)]
                    if n_pass == 3:
                        terms += [(whd_s, gl), (whl_s, gh)]
                    n_mm = 2 * len(terms)
                    j = 0
                    for (wt, xt) in terms:
                        for k in range(2):
                            nc.tensor.matmul(
                                pch_t[0:16, :],
                                lhsT=wt[:, k, 0:16],
                                rhs=xt[:, k, :],
                                start=(j == 0), stop=(j == n_mm - 1),
                                skip_group_check=True,
                            )
                            j += 1

                    # ---- copy heads/C to SBUF, transpose back to batch-major ----
                    ch_s = ep.tile([40, 512], F32, tag="chs")
                    nc.scalar.copy(ch_s, pch_t)
                    pbt_t = pbt.tile([128, 4, 40], F32, tag="pbt")
                    for c in range(4):
                        nc.tensor.matmul(
                            pbt_t[:, c, :],
                            lhsT=ch_s[0:40, c * 128:(c + 1) * 128],
                            rhs=id32[0:40, 0:40],
                            is_transpose=True,
                            start=(c == 0), stop=(c == 3),
                            skip_group_check=True,
                        )

                    # ---- per-row softmax / sigmoid / argmax-gather ----
                    mx = sp.tile([128, 4, 8], F32, tag="mx")
                    for c in range(4):
                        nc.vector.max(mx[:, c, :], pbt_t[:, c, 0:8])
                    ex = sp.tile([128, 4, 8], F32, tag="ex")
                    nc.scalar.activation(ex, pbt_t[:, :, 0:8], AF.Exp)
                    sums = sp.tile([128, 4, 1], F32, tag="sums")
                    nc.vector.tensor_reduce(
                        sums, ex, axis=mybir.AxisListType.X, op=OP.add
                    )
                    rsum = sp.tile([128, 4, 1], F32, tag="rsum")
                    nc.vector.reciprocal(rsum, sums)
                    probs_s = op_.tile([128, 4, 8], F32, tag="probs")
                    nc.vector.tensor_mul(probs_s, ex, rsum.to_broadcast([128, 4, 8]))
                    smoid_s = op_.tile([128, 4, 8], F32, tag="smoid")
                    nc.scalar.activation(smoid_s, pbt_t[:, :, 8:16], AF.Sigmoid)
                    mask = sp.tile([128, 4, 8], F32, tag="mask")
                    for c in range(4):
                        nc.vector.tensor_scalar(
                            mask[:, c, :], pbt_t[:, c, 0:8],
                            scalar1=mx[:, c, 0:1], scalar2=None,
                            op0=OP.is_equal,
                        )
                    mc = sp.tile([128, 4, 8], F32, tag="mc")
                    nc.vector.tensor_mul(mc, mask, pbt_t[:, :, 32:40])
                    outr = sp.tile([128, 4, 1], F32, tag="outr")
                    nc.vector.tensor_reduce(
                        outr, mc, axis=mybir.AxisListType.X, op=OP.add
                    )
                    outv = op_.tile([128, 4], F32, tag="outv")
                    nc.vector.tensor_scalar(
                        outv, outr.rearrange("p a b -> p (a b)"),
                        scalar1=bo_s[:, 0:1], scalar2=None, op0=OP.add,
                    )
                    top2_s = op_.tile([128, 4, 2], F32, tag="top2")
                    nc.vector.tensor_copy(top2_s, mx[:, :, 0:2])

                    # ---- output DMAs ----
                    nc.sync.dma_start(
                        out=bass.AP(
                            tensor=probs_d, offset=i * 4 * 8,
                            ap=[[rpp * 8, 128], [8, 4], [1, 8]],
                        ),
                        in_=probs_s,
                    )
                    nc.sync.dma_start(
                        out=bass.AP(
                            tensor=smoid_d, offset=i * 4 * 8,
                            ap=[[rpp * 8, 128], [8, 4], [1, 8]],
                        ),
                        in_=smoid_s,
                    )
                    nc.sync.dma_start(
                        out=bass.AP(
                            tensor=out_d, offset=i * 4,
                            ap=[[rpp, 128], [1, 4]],
                        ),
                        in_=outv,
                    )
                    nc.sync.dma_start(
                        out=bass.AP(
                            tensor=top2_d, offset=i * 4 * 2,
                            ap=[[rpp * 2, 128], [2, 4], [1, 2]],
                        ),
                        in_=top2_s,
                    )

            if rep == 1:
                body()
            else:
                with tc.For_i(0, rep, 1):
                    body()

    _split_multi_waits(nc)
    return nc


# ----------------------------------------------------------------------------
# Host-side orchestration
# ----------------------------------------------------------------------------

_RUNNERS = {}


def _runner(nrows, n_pass, rep=1):
    key = (nrows, n_pass, rep)
    if key not in _RUNNERS:
        nc = build_moe(nrows, n_pass, rep)
        _RUNNERS[key] = _make_runner(nc, N_CORES)
    return _RUNNERS[key]


def _split16(a):
    h = a.astype(np.float16)
    l = (a - h.astype(np.float32)).astype(np.float16)
    return h, l


def _weight_planes(W_fuse, W_a, W_smax, W_smoid, W_out):
    whd = np.zeros((256, 40), np.float32)
    whd[:, 0:8] = W_smax
    whd[:, 8:16] = W_smoid
    for k in range(8):
        whd[k * 32:(k + 1) * 32, 32 + k] = W_out[k * 32:(k + 1) * 32, 0]
    wfh, wfl = _split16(np.asarray(W_fuse, np.float32))
    wah, wal = _split16(np.asarray(W_a, np.float32))
    whh, whl = _split16(whd)
    return wfh, wfl, wah, wal, whh, whl


def _const_maps(W_fuse, b_fuse, W_a, b_a, W_smax, W_smoid, W_out, b_out, n_pass):
    wfh, wfl, wah, wal, whh, whl = _weight_planes(W_fuse, W_a, W_smax, W_smoid, W_out)
    consts = {
        "wfh": wfh, "wah": wah, "whd": whh,
        "bf": np.ascontiguousarray(
            np.asarray(b_fuse, np.float32).reshape(2, 128).T
        ),
        "ba": np.ascontiguousarray(
            np.asarray(b_a, np.float32).reshape(2, 128).T
        ),
        "bo": np.full((128, 1), np.float32(np.asarray(b_out).reshape(-1)[0])),
        "id16": np.eye(128, dtype=np.float16),
    }
    if n_pass == 3:
        consts.update({"wfl": wfl, "wal": wal, "whl": whl})
    return consts


def _run_sharded(call, nrows, per_core_arrays, consts):
    in_maps = []
    for c in range(N_CORES):
        m = dict(consts)
        for k, full in per_core_arrays.items():
            m[k] = full[c * nrows:(c + 1) * nrows]
        in_maps.append(m)
    res = call(in_maps)
    outs = {}
    for k in ("out", "probs", "smoid", "top2"):
        outs[k] = np.concatenate([r[k] for r in res], axis=0)
    return outs


def kernel(s_e, p_e, W_fuse, b_fuse, W_a, b_a,
           W_smax, b_smax, W_smoid, b_smoid, W_out, b_out):
    s_e = np.asarray(s_e, np.float32)
    p_e = np.asarray(p_e, np.float32)
    B = s_e.shape[0]
    assert B == B_TOTAL, f"kernel hardcoded for B={B_TOTAL}, got {B}"

    mode = os.environ.get("MOE_MODE", "repair")  # "repair" | "3pass"
    n_pass_bulk = 3 if mode == "3pass" else 1

    sh, sl = _split16(s_e)
    ph, pl = _split16(p_e)

    consts = _const_maps(
        W_fuse, b_fuse, W_a, b_a, W_smax, W_smoid, W_out, b_out, n_pass_bulk
    )
    call = _runner(R_BULK, n_pass_bulk)
    arrays = {"sh": sh, "ph": ph}
    if n_pass_bulk == 3:
        arrays.update({"sl": sl, "pl": pl})
    res = _run_sharded(call, R_BULK, arrays, consts)

    # device row layout -> natural row order
    # local row of (p, i, q) = p*rpp + 4i + q; outputs were written with
    # matching APs so res arrays are already in natural order.
    out = res["out"]
    probs = res["probs"]
    smoid = res["smoid"]

    if mode == "repair":
        gap = res["top2"][:, 0] - res["top2"][:, 1]
        flagged = np.flatnonzero(gap < THETA)
        if flagged.size:
            consts3 = _const_maps(
                W_fuse, b_fuse, W_a, b_a, W_smax, W_smoid, W_out, b_out, 3
            )
            call3 = _runner(REP_CAP, 3)
            cap_total = REP_CAP * N_CORES
            for lo in range(0, flagged.size, cap_total):
                idx = flagged[lo:lo + cap_total]
                pad = cap_total - idx.size
                idx_p = np.concatenate([idx, np.zeros(pad, np.int64)])
                g_s = np.ascontiguousarray(s_e[idx_p])
                g_p = np.ascontiguousarray(p_e[idx_p])
                gsh, gsl = _split16(g_s)
                gph, gpl = _split16(g_p)
                rres = _run_sharded(
                    call3, REP_CAP,
                    {"sh": gsh, "ph": gph, "sl": gsl, "pl": gpl},
                    consts3,
                )
                out[idx] = rres["out"][:idx.size]
                probs[idx] = rres["probs"][:idx.size]
                smoid[idx] = rres["smoid"][:idx.size]

    return out, probs, smoid
